# revision 2
# baseline (speedup 1.0000x reference)
"""3-layer GraphSAGE + classifier + log_softmax on 8 Trainium2 NeuronCores.

Self-contained: host-side sharding/packing + Bass/Tile device kernel.

Strategy
--------
concat([x, agg]) @ W  ==  x @ W_top + Ahat @ (x @ W_bot)   (linearity)
so aggregation happens in the 256-dim projected space.

- Nodes are permuted into 704 tiles of 128 (in-degree balanced), 88 tiles/core.
- Per layer: phase A computes r = x@W_top + b and p = x@W_bot per owned tile
  (PE matmuls, fp32r for layer 1, fp16 after), p is written fp16 and
  AllGathered so every core holds the full p table in DRAM.
- Phase B: per dst tile, gather p[src] rows for its in-edges via dma_gather
  (int16 indices -> three overlapping 32768-row windows), build a one-hot
  selection matrix S[e, d] = wn_e * (dst_local_e == d) on DVE, and accumulate
  agg = sum_c S_c.T @ msg_c on the PE into PSUM.  x_next = relu(agg + r).
- x_next is transposed on the PE (2x 128x128) to feed the next layer's
  stationary operand; the classifier (768->7) + log_softmax are fused into
  layer-3 phase B.
"""

import numpy as np

import concourse.bass as bass
import concourse.mybir as mybir
import concourse.tile as tile
from concourse import bacc
from concourse.bass_utils import run_bass_kernel_spmd
from concourse.masks import make_identity

# problem constants
N = 89250
IN_F = 500
HID = 256
NCLS = 7
FPAD = 512  # padded input feature dim

NC = 8  # cores
P = 128
NT = 704  # node tiles
TPC = NT // NC  # 88 tiles per core
NPAD = NT * P  # 90112
NPC = TPC * P  # 11264 nodes per core
G = 2  # tiles per gather group
NGRP = TPC // G

WBASE = (0, 28672, 57344)  # gather window base rows
WCAP = 32768  # int16 index reach

f32 = mybir.dt.float32
f32r = mybir.dt.float32r
f16 = mybir.dt.float16
i16 = mybir.dt.int16
i32 = mybir.dt.int32

_compile_cache = {}


# --------------------------------------------------------------------------
# host-side prep
# --------------------------------------------------------------------------

def _assign_tiles(in_deg):
    """LPT: assign node ids (0..NPAD) to (tile, slot), balancing in-edges."""
    import heapq

    order = np.argsort(-in_deg, kind="stable")
    heap = [(0, t) for t in range(NT)]
    heapq.heapify(heap)
    counts = np.zeros(NT, np.int32)
    newpos = np.empty(NPAD, np.int64)
    for v in order:
        load, t = heapq.heappop(heap)
        newpos[v] = t * P + counts[t]
        counts[t] += 1
        if counts[t] < P:
            heapq.heappush(heap, (load + int(in_deg[v]), t))
    return newpos


def _window_split(s2_t):
    """Split one tile's edge src ids into 3 windows; returns list of 3 arrays
    of edge positions (indices into s2_t)."""
    z = s2_t
    hard0 = z < WBASE[1]
    flex01 = (z >= WBASE[1]) & (z < WCAP)
    hard1 = (z >= WCAP) & (z < WBASE[2])
    flex12 = (z >= WBASE[2]) & (z < WBASE[1] + WCAP)
    hard2 = z >= WBASE[1] + WCAP
    n = len(z)
    tgt = n / 3.0
    n0, n4 = int(hard0.sum()), int(hard2.sum())
    a = int(np.clip(round(tgt - n0), 0, int(flex01.sum())))
    c = int(np.clip(round(tgt - n4), 0, int(flex12.sum())))
    i_f01 = np.nonzero(flex01)[0]
    i_f12 = np.nonzero(flex12)[0]
    w0 = np.concatenate([np.nonzero(hard0)[0], i_f01[:a]])
    w1 = np.concatenate([i_f01[a:], np.nonzero(hard1)[0], i_f12[c:]])
    w2 = np.concatenate([np.nonzero(hard2)[0], i_f12[:c]])
    return [w0, w1, w2]


def prep(x, edge_index, edge_weight):
    src = edge_index[0].astype(np.int64)
    dst = edge_index[1].astype(np.int64)
    ew = edge_weight.astype(np.float32)

    cnt = np.bincount(dst, minlength=N).astype(np.float32)
    wn = ew / np.maximum(cnt[dst], 1.0)

    in_deg = np.zeros(NPAD, np.int64)
    in_deg[:N] = np.bincount(dst, minlength=N)
    newpos = _assign_tiles(in_deg)

    s2 = newpos[src]
    d2 = newpos[dst]
    tile_of = d2 // P
    dl = (d2 % P).astype(np.float32)

    # per-tile edge lists
    order = np.argsort(tile_of, kind="stable")
    s2o, dlo, wno, tso = s2[order], dl[order], wn[order], tile_of[order]
    starts = np.searchsorted(tso, np.arange(NT + 1))

    # first pass: window split per tile, find quotas
    tile_windows = []
    bucket_sizes = np.zeros((NT, 3), np.int64)
    for t in range(NT):
        lo, hi = starts[t], starts[t + 1]
        wsplit = _window_split(s2o[lo:hi])
        tile_windows.append(wsplit)
        for w in range(3):
            bucket_sizes[t, w] = len(wsplit[w])
    Kw = [int(np.ceil(bucket_sizes[:, w].max() / P)) for w in range(3)]
    Kw = [max(k, 1) for k in Kw]
    K = sum(Kw)
    offw = [0, Kw[0], Kw[0] + Kw[1]]

    # second pass: pack slots
    meta_dw = np.zeros((NT, P, 2 * K), np.float16)
    # gather index lists: [NT, 3] ragged -> per (tile, w): int16 [Kw*P]
    gl = [np.zeros((NT, Kw[w] * P), np.int16) for w in range(3)]
    for t in range(NT):
        lo = starts[t]
        for w in range(3):
            pos = tile_windows[t][w]
            nw = len(pos)
            idxs = (s2o[lo + pos] - WBASE[w]).astype(np.int16)
            assert (idxs >= 0).all() and (idxs < WCAP).all()
            gl[w][t, :nw] = idxs
            sl = np.arange(nw)
            ch = sl // P
            pp = sl % P
            meta_dw[t, pp, offw[w] + ch] = dlo[lo + pos]
            meta_dw[t, pp, K + offw[w] + ch] = wno[lo + pos]

    # wrap gather lists into per-(group, window) 16-partition layout
    GI_COLS = G * K * 8
    gidx = np.zeros((NT // G, P, GI_COLS), np.int16)
    for g in range(NT // G):
        col = 0
        for w in range(3):
            seg = gl[w][g * G:(g + 1) * G].reshape(-1)  # [G*Kw*P]
            wrapped = seg.reshape(-1, 16).T  # [16, G*Kw*8]
            gidx[g, :, col:col + wrapped.shape[1]] = np.tile(wrapped, (8, 1))
            col += wrapped.shape[1]

    # transposed, padded, permuted node features
    xT = np.zeros((FPAD, NPAD), np.float16)
    xT[:IN_F, newpos[:N]] = x.T

    return {
        "newpos": newpos,
        "K": K,
        "Kw": tuple(Kw),
        "xT": xT,
        "meta_dw": meta_dw,
        "gidx": gidx,
    }


def pack_weights(W1, b1, W2, b2, W3, b3, Wl, bl):
    def chunk_rhs(W, kchunks, dtype):
        # [F, 512] -> [128, kchunks, 512]
        F = W.shape[0]
        Wp = np.zeros((kchunks * P, 512), np.float32)
        Wp[:F] = W
        return np.ascontiguousarray(
            Wp.reshape(kchunks, P, 512).transpose(1, 0, 2)
        ).astype(dtype)

    w1cat = np.concatenate([W1[:IN_F], W1[IN_F:]], axis=1)  # [500, 512]
    w2cat = np.concatenate([W2[:HID], W2[HID:]], axis=1)  # [256, 512]
    w3cat = np.concatenate([W3[:HID], W3[HID:]], axis=1)
    wl = np.ascontiguousarray(
        Wl.reshape(6, P, NCLS).transpose(1, 0, 2)
    ).astype(np.float16)  # [128, 6, 7]
    return {
        "w1": chunk_rhs(w1cat, 4, np.float16),
        "w2": chunk_rhs(w2cat, 2, np.float16),
        "w3": chunk_rhs(w3cat, 2, np.float16),
        "wl": wl,
        "b1": np.tile(b1[None, :], (P, 1)).astype(np.float32),
        "b2": np.tile(b2[None, :], (P, 1)).astype(np.float32),
        "b3": np.tile(b3[None, :], (P, 1)).astype(np.float32),
        "bl": np.tile(bl[None, :], (P, 1)).astype(np.float32),
    }


# --------------------------------------------------------------------------
# device kernel
# --------------------------------------------------------------------------

def build(K, Kw, stage="full", repeat=1):
    """stage: 'a1' (phase A L1), 'ag1' (+AllGather), 'b1' (+phase B L1),
    'l2' (2 layers), 'full'.  repeat: run the whole pipeline N times
    (for differential timing)."""
    GI_COLS = G * K * 8
    offw = [0, Kw[0], Kw[0] + Kw[1]]

    nc = bacc.Bacc("TRN2", target_bir_lowering=False, debug=False, num_devices=NC)

    xTc = nc.dram_tensor("xTc", [FPAD, NPC], f16, kind="ExternalInput")
    meta_dw_d = nc.dram_tensor("meta_dw", [TPC, P, 2 * K], f16, kind="ExternalInput")
    gidx_d = nc.dram_tensor("gidx", [NGRP, P, GI_COLS], i16, kind="ExternalInput")
    w1_d = nc.dram_tensor("w1", [P, 4, 512], f16, kind="ExternalInput")
    w2_d = nc.dram_tensor("w2", [P, 2, 512], f16, kind="ExternalInput")
    w3_d = nc.dram_tensor("w3", [P, 2, 512], f16, kind="ExternalInput")
    wl_d = nc.dram_tensor("wl", [P, 6, NCLS], f16, kind="ExternalInput")
    b1_d = nc.dram_tensor("b1", [P, HID], f32, kind="ExternalInput")
    b2_d = nc.dram_tensor("b2", [P, HID], f32, kind="ExternalInput")
    b3_d = nc.dram_tensor("b3", [P, HID], f32, kind="ExternalInput")
    bl_d = nc.dram_tensor("bl", [P, NCLS], f32, kind="ExternalInput")
    out_d = nc.dram_tensor("out", [NPC, NCLS], f32, kind="ExternalOutput")

    with tile.TileContext(nc) as tc:
        with (
            tc.tile_pool(name="dram", bufs=1, space="DRAM") as dram,
            tc.tile_pool(name="const", bufs=1) as cpool,
            tc.tile_pool(name="lx", bufs=6) as lxpool,
            tc.tile_pool(name="stage", bufs=3) as stpool,
            tc.tile_pool(name="msg", bufs=2) as msgpool,
            tc.tile_pool(name="sbuild", bufs=2) as sbpool,
            tc.tile_pool(name="psa", bufs=2, space="PSUM") as psa,
            tc.tile_pool(name="psagg", bufs=2, space="PSUM") as psagg,
            tc.tile_pool(name="pstr", bufs=2, space="PSUM") as pstr,
            tc.tile_pool(name="pscls", bufs=2, space="PSUM") as pscls,
        ):
            # ---- DRAM intermediates (allocated per repeat)
            def alloc_inter(rep):
                pl = [dram.tile([NPC, HID], f16, name=f"p{i}loc{rep}")
                      for i in range(3)]
                pf = [dram.tile([NPAD, HID], f16, addr_space="Shared",
                                name=f"p{i}full{rep}") for i in range(3)]
                rd = [dram.tile([NPC, HID], f16, name=f"r{i}d{rep}")
                      for i in range(3)]
                x2 = dram.tile([TPC, 2, P, P], f16, name=f"x2td{rep}")
                return pl, pf, rd, x2

            p_loc, p_full, r_dram, x2t_dram = alloc_inter(0)

            # ---- constants
            w1_sb = cpool.tile([P, 4, 512], f16)
            nc.sync.dma_start(w1_sb[:], w1_d[:])
            w2_sb = cpool.tile([P, 2, 512], f16)
            nc.sync.dma_start(w2_sb[:], w2_d[:])
            w3_sb = cpool.tile([P, 2, 512], f16)
            nc.sync.dma_start(w3_sb[:], w3_d[:])
            wl_sb = cpool.tile([P, 6, NCLS], f16)
            nc.sync.dma_start(wl_sb[:], wl_d[:])
            b_sb = []
            for name, t in (("b1", b1_d), ("b2", b2_d), ("b3", b3_d)):
                bt = cpool.tile([P, HID], f32, name=name + "sb")
                nc.sync.dma_start(bt[:], t[:])
                b_sb.append(bt)
            bl_sb = cpool.tile([P, NCLS], f32)
            nc.sync.dma_start(bl_sb[:], bl_d[:])

            meta_sb = cpool.tile([P, TPC, 2 * K], f16)
            nc.sync.dma_start(
                meta_sb[:],
                meta_dw_d[:].rearrange("t p c -> p t c"),
            )
            gidx_sb = cpool.tile([P, NGRP, GI_COLS], i16)
            nc.sync.dma_start(
                gidx_sb[:],
                gidx_d[:].rearrange("g p c -> p g c"),
            )

            iota_i = cpool.tile([P, P], i32)
            nc.gpsimd.iota(iota_i[:], pattern=[[1, P]], base=0, channel_multiplier=0)
            iota_f = cpool.tile([P, P], f16)
            nc.vector.tensor_copy(iota_f[:], iota_i[:])
            ident = cpool.tile([P, P], f16)
            make_identity(nc, ident[:])

            x1t_sb = cpool.tile([P, TPC, 2, P], f16)
            out_sb = cpool.tile([P, TPC, NCLS], f32)

            # ---- phase A: r/p for one layer (batched per 4-tile group)
            def grp_rows(buf, g):
                return buf[g * G * P:(g + 1) * G * P, :].rearrange(
                    "(t p) c -> p t c", p=P
                )

            def grp_blocks(buf, g):
                return buf[g * G:(g + 1) * G].rearrange("t h p q -> p (t h) q")

            def phase_a(layer):
                for g in range(NGRP):
                    if layer == 0:
                        lxs = []
                        for k in range(4):
                            lx = lxpool.tile([P, G * P], f16, name="lx", tag="lx")
                            nc.sync.dma_start(
                                lx[:],
                                xTc[k * P:(k + 1) * P, g * G * P:(g + 1) * G * P],
                            )
                            lxs.append(lx)
                    elif layer == 2:
                        x2s = lxpool.tile([P, 2 * G, P], f16, name="x2s", tag="x2s")
                        nc.sync.dma_start(x2s[:], grp_blocks(x2t_dram, g))
                    rst = stpool.tile([P, G, HID], f16, name="rst", tag="rst")
                    pst = stpool.tile([P, G, HID], f16, name="pst", tag="pst")
                    for gt in range(G):
                        t = g * G + gt
                        ps = psa.tile([P, 512], f32, name="psA", tag="psA")
                        if layer == 0:
                            for k in range(4):
                                nc.tensor.matmul(
                                    out=ps[:], lhsT=lxs[k][:, gt * P:(gt + 1) * P],
                                    rhs=w1_sb[:, k, :],
                                    start=(k == 0), stop=(k == 3),
                                )
                        elif layer == 1:
                            for k in range(2):
                                nc.tensor.matmul(
                                    out=ps[:], lhsT=x1t_sb[:, t, k, :],
                                    rhs=w2_sb[:, k, :],
                                    start=(k == 0), stop=(k == 1),
                                )
                        else:
                            for k in range(2):
                                nc.tensor.matmul(
                                    out=ps[:], lhsT=x2s[:, gt * 2 + k, :],
                                    rhs=w3_sb[:, k, :],
                                    start=(k == 0), stop=(k == 1),
                                )
                        nc.vector.tensor_tensor(
                            out=rst[:, gt, :], in0=ps[:, :HID], in1=b_sb[layer][:],
                            op=mybir.AluOpType.add,
                        )
                        nc.vector.tensor_copy(pst[:, gt, :], ps[:, HID:])
                    nc.sync.dma_start(grp_rows(r_dram[layer], g), rst[:])
                    nc.sync.dma_start(grp_rows(p_loc[layer], g), pst[:])

            # ---- phase B: aggregate + relu (+ classifier on last layer)
            def phase_b(layer, sub="full"):
                last = layer == 2
                for g in range(NGRP):
                    msgs = []
                    col = 0
                    for w in range(3):
                        ncols = G * Kw[w] * 8
                        m = msgpool.tile(
                            [P, G * Kw[w], HID], f16, name=f"m{w}", tag=f"m{w}"
                        )
                        nc.gpsimd.dma_gather(
                            out_ap=m[:],
                            in_ap=p_full[layer][WBASE[w]:, :],
                            idxs_ap=gidx_sb[:, g, col:col + ncols],
                            num_idxs=G * Kw[w] * P,
                            num_idxs_reg=G * Kw[w] * P,
                            elem_size=HID,
                            single_packet=(G * Kw[w] * P <= 1024),
                        )
                        msgs.append(m)
                        col += ncols
                    rst = stpool.tile([P, G, HID], f16, name="rl", tag="rl")
                    nc.sync.dma_start(rst[:], grp_rows(r_dram[layer], g))
                    if layer == 1:
                        x2w = stpool.tile([P, 2 * G, P], f16, name="x2w", tag="x2w")
                    if last:
                        x2c = lxpool.tile([P, 2 * G, P], f16, name="x2c", tag="x2c")
                        nc.sync.dma_start(x2c[:], grp_blocks(x2t_dram, g))
                        shst = stpool.tile([P, G, NCLS], f32, name="shst", tag="shst")
                        smst = stpool.tile([P, G], f32, name="smst", tag="smst")
                    for gt in range(G):
                        t = g * G + gt
                        if sub == "gather":
                            nc.vector.tensor_copy(
                                out_sb[:, t, :], msgs[0][:, gt * Kw[0], :NCLS]
                            )
                            continue
                        s_all = sbpool.tile([P, K, P], f16, name="sall", tag="sall")
                        nc.vector.tensor_tensor(
                            out=s_all[:],
                            in0=meta_sb[:, t, :K].unsqueeze(2).broadcast_to([P, K, P]),
                            in1=iota_f[:].unsqueeze(1).broadcast_to([P, K, P]),
                            op=mybir.AluOpType.is_equal,
                        )
                        nc.vector.tensor_tensor(
                            out=s_all[:],
                            in0=s_all[:],
                            in1=meta_sb[:, t, K:2 * K].unsqueeze(2).broadcast_to(
                                [P, K, P]
                            ),
                            op=mybir.AluOpType.mult,
                        )
                        if sub == "sbuild":
                            nc.vector.tensor_copy(
                                out_sb[:, t, :], s_all[:, 0, :NCLS]
                            )
                            continue
                        agg = psagg.tile([P, HID], f32, name="agg", tag="agg")
                        c = 0
                        for w in range(3):
                            for j in range(Kw[w]):
                                nc.tensor.matmul(
                                    out=agg[:],
                                    lhsT=s_all[:, c, :],
                                    rhs=msgs[w][:, gt * Kw[w] + j, :],
                                    start=(c == 0),
                                    stop=(c == K - 1),
                                )
                                c += 1
                        if sub == "mm":
                            nc.vector.tensor_copy(out_sb[:, t, :], agg[:, :NCLS])
                            continue
                        xsum = stpool.tile([P, HID], f16, name="xsum", tag="xsum")
                        nc.vector.tensor_tensor(
                            out=xsum[:], in0=agg[:], in1=rst[:, gt, :],
                            op=mybir.AluOpType.add,
                        )
                        xn = stpool.tile([P, HID], f16, name="xn", tag="xn")
                        nc.vector.tensor_scalar_max(xn[:], xsum[:], 0.0)
                        x3t = []
                        for h in range(2):
                            tp = pstr.tile([P, P], f16, name="tp", tag="tp")
                            nc.tensor.transpose(
                                out=tp[:], in_=xn[:, h * P:(h + 1) * P],
                                identity=ident[:],
                            )
                            if layer == 0:
                                nc.vector.tensor_copy(x1t_sb[:, t, h, :], tp[:])
                            elif layer == 1:
                                nc.vector.tensor_copy(x2w[:, gt * 2 + h, :], tp[:])
                            else:
                                xt = stpool.tile([P, P], f16, name="x3t", tag="x3t")
                                nc.vector.tensor_copy(xt[:], tp[:])
                                x3t.append(xt)
                        if last:
                            # classifier: 6 k-chunks of 128
                            cls = pscls.tile([P, NCLS], f32, name="cls", tag="cls")
                            chunks = [
                                x1t_sb[:, t, 0, :], x1t_sb[:, t, 1, :],
                                x2c[:, gt * 2, :], x2c[:, gt * 2 + 1, :],
                                x3t[0][:], x3t[1][:],
                            ]
                            for kk in range(6):
                                nc.tensor.matmul(
                                    out=cls[:], lhsT=chunks[kk], rhs=wl_sb[:, kk, :],
                                    start=(kk == 0), stop=(kk == 5),
                                )
                            lg = stpool.tile([P, NCLS], f32, name="lg", tag="lg")
                            nc.vector.tensor_tensor(
                                out=lg[:], in0=cls[:], in1=bl_sb[:],
                                op=mybir.AluOpType.add,
                            )
                            mx = stpool.tile([P, 1], f32, name="mx", tag="mx")
                            nc.vector.tensor_reduce(
                                out=mx[:], in_=lg[:], axis=mybir.AxisListType.X,
                                op=mybir.AluOpType.max,
                            )
                            nc.vector.tensor_scalar(
                                out=shst[:, gt, :], in0=lg[:], scalar1=mx[:, :1],
                                scalar2=None, op0=mybir.AluOpType.subtract,
                            )
                    if layer == 1 and sub == "full":
                        nc.sync.dma_start(grp_blocks(x2t_dram, g), x2w[:])
                    if last and sub == "full":
                        ex = stpool.tile([P, G, NCLS], f32, name="ex", tag="ex")
                        nc.scalar.activation(
                            ex[:], shst[:], mybir.ActivationFunctionType.Exp
                        )
                        nc.vector.tensor_reduce(
                            out=smst[:], in_=ex[:], axis=mybir.AxisListType.X,
                            op=mybir.AluOpType.add,
                        )
                        lsm = stpool.tile([P, G], f32, name="lsm", tag="lsm")
                        nc.scalar.activation(
                            lsm[:], smst[:], mybir.ActivationFunctionType.Ln
                        )
                        for gt in range(G):
                            t = g * G + gt
                            nc.vector.tensor_scalar(
                                out=out_sb[:, t, :], in0=shst[:, gt, :],
                                scalar1=lsm[:, gt:gt + 1],
                                scalar2=None, op0=mybir.AluOpType.subtract,
                            )

            if stage != "full":
                nc.gpsimd.memset(out_sb[:], 0.0)
            nlayers = {"a1": 1, "ag1": 1, "b1": 1, "l2": 2}.get(stage, 3)
            bsub = {"b1g": "gather", "b1s": "sbuild", "b1m": "mm",
                    "bg3": "gather", "bs3": "sbuild", "bm3": "mm",
                    "b1r": "relu"}.get(stage, "full")
            do_ag = stage not in ("a1", "a3", "noag")
            do_b = stage not in ("a1", "a3", "ag1", "ag3")
            if not do_b or bsub != "full":
                # timing-only variants: initialize tensors phase B would write
                nc.gpsimd.memset(x1t_sb[:], 0.0)
                nc.sync.dma_start(
                    x2t_dram[:].rearrange("t h p q -> p (t h) q"),
                    x1t_sb[:].rearrange("p t h q -> p (t h) q"),
                )
            for _rep in range(repeat):
                if _rep > 0:
                    p_loc, p_full, r_dram, x2t_dram = alloc_inter(_rep)
                for layer in range(nlayers):
                    phase_a(layer)
                    if do_ag:
                        nc.gpsimd.collective_compute(
                            "AllGather",
                            mybir.AluOpType.bypass,
                            replica_groups=[list(range(NC))],
                            ins=[p_loc[layer].opt()],
                            outs=[p_full[layer].opt()],
                        )
                    if do_b:
                        phase_b(layer, sub=bsub)

            nc.sync.dma_start(
                out_d[:].rearrange("(t p) j -> p t j", p=P), out_sb[:]
            )

    nc.compile()
    return nc


# --------------------------------------------------------------------------
# entry point
# --------------------------------------------------------------------------

def make_in_maps(pp, wts):
    in_maps = []
    for c in range(NC):
        in_maps.append({
            "xTc": np.ascontiguousarray(pp["xT"][:, c * NPC:(c + 1) * NPC]),
            "meta_dw": np.ascontiguousarray(pp["meta_dw"][c * TPC:(c + 1) * TPC]),
            "gidx": np.ascontiguousarray(pp["gidx"][c * NGRP:(c + 1) * NGRP]),
            **wts,
        })
    return in_maps


def kernel(x, edge_index, edge_weight, W1, b1, W2, b2, W3, b3, Wl, bl):
    x = np.asarray(x, dtype=np.float32)
    edge_index = np.asarray(edge_index)
    edge_weight = np.asarray(edge_weight, dtype=np.float32)

    pp = prep(x, edge_index, edge_weight)
    K, Kw = pp["K"], pp["Kw"]
    wts = pack_weights(
        np.asarray(W1, np.float32), np.asarray(b1, np.float32),
        np.asarray(W2, np.float32), np.asarray(b2, np.float32),
        np.asarray(W3, np.float32), np.asarray(b3, np.float32),
        np.asarray(Wl, np.float32), np.asarray(bl, np.float32),
    )

    key = (K, Kw)
    if key not in _compile_cache:
        _compile_cache[key] = build(K, Kw)
    nc = _compile_cache[key]

    in_maps = make_in_maps(pp, wts)

    res = run_bass_kernel_spmd(nc, in_maps, list(range(NC)))
    out_full = np.concatenate([res.results[c]["out"] for c in range(NC)], axis=0)
    return out_full[pp["newpos"][:N]].astype(np.float32)


if __name__ == "__main__":
    import time

    rng = np.random.default_rng(0)
    # tiny self-check of prep packing invariants on random data
    E = 899756
    ei = rng.integers(0, N, (2, E)).astype(np.int32)
    ew = rng.random(E, dtype=np.float32)
    x = rng.standard_normal((N, IN_F), dtype=np.float32)
    t0 = time.time()
    pp = prep(x, ei, ew)
    print("prep", time.time() - t0, "K =", pp["K"], "Kw =", pp["Kw"])



# revision 11
# speedup vs baseline: 1.5340x; 1.5340x over previous
"""3-layer GraphSAGE + classifier + log_softmax on 8 Trainium2 NeuronCores.

Self-contained: host-side sharding/packing + Bass/Tile device kernel.

Strategy
--------
concat([x, agg]) @ W  ==  x @ W_top + Ahat @ (x @ W_bot)   (linearity)
so aggregation happens in the 256-dim projected space.

- Nodes are permuted into 704 tiles of 128 (in-degree balanced), 88 tiles/core.
- Per layer, phase A computes p = x@W_bot per owned tile (for layer 1 the full
  [r|p] 512-wide product, with b1 folded in via a constant-1 input row), p is
  written fp16 and AllGathered so every core holds the full p table in DRAM.
- Phase B: per dst tile, gather p[src] rows for its in-edges via dma_gather
  (int16 indices -> four overlapping 32768-row windows with a uniform
  chunks-per-window signature so every tile has exactly K chunks of 128),
  build a one-hot selection matrix S[e, d] = wn_e * (dst_local_e == d) on DVE,
  and accumulate on the PE into one PSUM tile:
      agg = ident@r (or ident@bias) + x@W_top (layers 2,3) + sum_c S_c.T@msg_c
  x_next = relu(PSUM) on the scalar engine.
- x_next is transposed on the PE (2x 128x128) to feed the next layer's
  stationary operand; x1^T and x2^T live entirely in SBUF; the classifier
  (768->7) + log_softmax are fused into layer-3 phase B.
"""

import numpy as np

import concourse.bass as bass
import concourse.mybir as mybir
import concourse.tile as tile
from concourse import bacc
from concourse.bass_utils import run_bass_kernel_spmd
from concourse.masks import make_identity

# problem constants
N = 89250
IN_F = 500
HID = 256
NCLS = 7
FPAD = 512  # padded input feature dim

NC = 8  # cores
P = 128
NT = 704  # node tiles
TPC = NT // NC  # 88 tiles per core
NPAD = NT * P  # 90112
NPC = TPC * P  # 11264 nodes per core
G = 4  # tiles per gather group
NGRP = TPC // G  # 22

NW = 4
WBASE = (0, 19456, 38912, 57344)  # gather window base rows
WCAP = 32768  # int16 index reach

f32 = mybir.dt.float32
f16 = mybir.dt.float16
i16 = mybir.dt.int16
i32 = mybir.dt.int32

_compile_cache = {}


# --------------------------------------------------------------------------
# host-side prep
# --------------------------------------------------------------------------

def _assign_tiles(in_deg):
    """LPT: assign node ids (0..NPAD) to (tile, slot), balancing in-edges."""
    import heapq

    order = np.argsort(-in_deg, kind="stable")
    heap = [(0, t) for t in range(NT)]
    heapq.heapify(heap)
    counts = np.zeros(NT, np.int32)
    newpos = np.empty(NPAD, np.int64)
    for v in order:
        load, t = heapq.heappop(heap)
        newpos[v] = t * P + counts[t]
        counts[t] += 1
        if counts[t] < P:
            heapq.heappush(heap, (load + int(in_deg[v]), t))
    return newpos


def _window_split(z, sig):
    """Split one tile's edge src rows (global) into NW window buckets with
    bucket w capped at 128*sig[w]. Returns list of NW arrays of edge
    positions (indices into z), or None if infeasible under sig."""
    caps = [128 * s for s in sig]
    n = len(z)
    # region of each edge: hard-w or flex-w/w+1
    buckets = []
    avail = np.arange(n)
    zl = z
    carry = np.array([], dtype=np.int64)
    for w in range(NW):
        lo = WBASE[w]
        hi_excl = WBASE[w + 1] if w + 1 < NW else NPAD
        reach = WBASE[w] + WCAP
        # edges that MUST be in window <= w: rows < next window's base
        in_hard = avail[(z[avail] >= lo) & (z[avail] < hi_excl)]
        must = np.concatenate([carry, in_hard])
        if w + 1 < NW:
            flex = avail[(z[avail] >= hi_excl) & (z[avail] < reach)]
        else:
            flex = np.array([], dtype=np.int64)
        if len(must) > caps[w]:
            return None
        take_flex = min(caps[w] - len(must), len(flex))
        bucket = np.concatenate([must, flex[:take_flex]])
        carry = flex[take_flex:]
        buckets.append(bucket.astype(np.int64))
        avail = avail[z[avail] >= reach]
    if len(carry):
        return None
    return buckets


def prep(x, edge_index, edge_weight):
    src = edge_index[0].astype(np.int64)
    dst = edge_index[1].astype(np.int64)
    ew = edge_weight.astype(np.float32)

    cnt = np.bincount(dst, minlength=N).astype(np.float32)
    wn = ew / np.maximum(cnt[dst], 1.0)

    in_deg = np.zeros(NPAD, np.int64)
    in_deg[:N] = np.bincount(dst, minlength=N)
    newpos = _assign_tiles(in_deg)

    s2 = newpos[src]
    d2 = newpos[dst]
    tile_of = d2 // P
    dl = (d2 % P).astype(np.float32)

    # per-tile edge lists
    order = np.argsort(tile_of, kind="stable")
    s2o, dlo, wno, tso = s2[order], dl[order], wn[order], tile_of[order]
    starts = np.searchsorted(tso, np.arange(NT + 1))

    # find a uniform chunks-per-window signature that fits every tile
    sig = [3, 2, 2, 3]
    for _tries in range(8):
        tile_windows = []
        ok = True
        for t in range(NT):
            ws = _window_split(s2o[starts[t]:starts[t + 1]], sig)
            if ws is None:
                ok = False
                break
            tile_windows.append(ws)
        if ok:
            break
        sig = [s + 1 for s in sig]
    else:
        raise RuntimeError("window split failed")
    Kw = tuple(sig)
    K = sum(Kw)
    offw = [sum(Kw[:w]) for w in range(NW)]

    # pack slots
    meta_dw = np.zeros((NT, P, 2 * K), np.float16)
    gl = [np.zeros((NT, Kw[w] * P), np.int16) for w in range(NW)]
    for t in range(NT):
        lo = starts[t]
        for w in range(NW):
            pos = tile_windows[t][w]
            nw = len(pos)
            idxs = (s2o[lo + pos] - WBASE[w]).astype(np.int16)
            assert (idxs >= 0).all() and (idxs < WCAP).all()
            gl[w][t, :nw] = idxs
            sl = np.arange(nw)
            ch = sl // P
            pp = sl % P
            meta_dw[t, pp, offw[w] + ch] = dlo[lo + pos]
            meta_dw[t, pp, K + offw[w] + ch] = wno[lo + pos]

    # wrap gather lists into per-(group, window) 16-partition layout
    GI_COLS = G * K * 8
    gidx = np.zeros((NT // G, P, GI_COLS), np.int16)
    for g in range(NT // G):
        col = 0
        for w in range(NW):
            seg = gl[w][g * G:(g + 1) * G].reshape(-1)  # [G*Kw*P]
            wrapped = seg.reshape(-1, 16).T  # [16, G*Kw*8]
            gidx[g, :, col:col + wrapped.shape[1]] = np.tile(wrapped, (8, 1))
            col += wrapped.shape[1]

    # transposed, padded, permuted node features; row IN_F = 1.0 feeds the
    # folded b1 (see pack_weights)
    xT = np.zeros((FPAD, NPAD), np.float16)
    xT[:IN_F, newpos[:N]] = x.T
    xT[IN_F, :] = 1.0

    return {
        "newpos": newpos,
        "K": K,
        "Kw": Kw,
        "xT": xT,
        "meta_dw": meta_dw,
        "gidx": gidx,
    }


def pack_weights(W1, b1, W2, b2, W3, b3, Wl, bl):
    def chunk_rhs(W, kchunks, dtype):
        # [F, 512] -> [128, kchunks, 512]
        F = W.shape[0]
        Wp = np.zeros((kchunks * P, 512), np.float32)
        Wp[:F] = W
        return np.ascontiguousarray(
            Wp.reshape(kchunks, P, 512).transpose(1, 0, 2)
        ).astype(dtype)

    w1cat = np.concatenate([W1[:IN_F], W1[IN_F:]], axis=1)  # [500, 512]
    w1cat = np.concatenate(
        [w1cat, np.concatenate([b1, np.zeros(HID, np.float32)])[None, :]]
    )  # row 500 = [b1 | 0]; xT row 500 is 1.0
    w2cat = np.concatenate([W2[:HID], W2[HID:]], axis=1)  # [256, 512]
    w3cat = np.concatenate([W3[:HID], W3[HID:]], axis=1)
    wl = np.ascontiguousarray(
        Wl.reshape(6, P, NCLS).transpose(1, 0, 2)
    ).astype(np.float16)  # [128, 6, 7]
    return {
        "w1": chunk_rhs(w1cat, 4, np.float16),
        "w2": chunk_rhs(w2cat, 2, np.float16),
        "w3": chunk_rhs(w3cat, 2, np.float16),
        "wl": wl,
        "b2": np.tile(b2[None, :], (P, 1)).astype(np.float16),
        "b3": np.tile(b3[None, :], (P, 1)).astype(np.float16),
        "bl": np.tile(bl[None, :], (P, 1)).astype(np.float16),
    }


# --------------------------------------------------------------------------
# device kernel
# --------------------------------------------------------------------------

def build(K, Kw, stage="full", repeat=1):
    """stage: 'a1' (phase A L1), 'ag1' (+AllGather), 'b1g' (+gathers only),
    'b1' (full layer 1), 'full'."""
    GI_COLS = G * K * 8
    offw = [sum(Kw[:w]) for w in range(NW)]

    nc = bacc.Bacc("TRN2", target_bir_lowering=False, debug=False, num_devices=NC,
                   num_swdge_queues=4)

    xTc = nc.dram_tensor("xTc", [FPAD, NPC], f16, kind="ExternalInput")
    meta_dw_d = nc.dram_tensor("meta_dw", [TPC, P, 2 * K], f16, kind="ExternalInput")
    gidx_d = nc.dram_tensor("gidx", [NGRP, P, GI_COLS], i16, kind="ExternalInput")
    w1_d = nc.dram_tensor("w1", [P, 4, 512], f16, kind="ExternalInput")
    w2_d = nc.dram_tensor("w2", [P, 2, 512], f16, kind="ExternalInput")
    w3_d = nc.dram_tensor("w3", [P, 2, 512], f16, kind="ExternalInput")
    wl_d = nc.dram_tensor("wl", [P, 6, NCLS], f16, kind="ExternalInput")
    b2_d = nc.dram_tensor("b2", [P, HID], f16, kind="ExternalInput")
    b3_d = nc.dram_tensor("b3", [P, HID], f16, kind="ExternalInput")
    bl_d = nc.dram_tensor("bl", [P, NCLS], f16, kind="ExternalInput")
    out_d = nc.dram_tensor("out", [NPC, NCLS], f32, kind="ExternalOutput")

    with tile.TileContext(nc) as tc:
        with (
            tc.tile_pool(name="dram", bufs=1, space="DRAM") as dram,
            tc.tile_pool(name="const", bufs=1) as cpool,
            tc.tile_pool(name="lx", bufs=8) as lxpool,
            tc.tile_pool(name="stage", bufs=3) as stpool,
            tc.tile_pool(name="msg", bufs=2) as msgpool,
            tc.tile_pool(name="sbuild", bufs=2) as sbpool,
            tc.tile_pool(name="psa", bufs=2, space="PSUM") as psa,
            tc.tile_pool(name="psagg", bufs=2, space="PSUM") as psagg,
            tc.tile_pool(name="pstr", bufs=2, space="PSUM") as pstr,
            tc.tile_pool(name="pscls", bufs=2, space="PSUM") as pscls,
        ):
            # ---- DRAM intermediates (allocated per repeat)
            def alloc_inter(rep):
                pl = [dram.tile([NPC, HID], f16, name=f"p{i}loc{rep}")
                      for i in range(3)]
                pf = [dram.tile([NPAD, HID], f16, addr_space="Shared",
                                name=f"p{i}full{rep}") for i in range(3)]
                return pl, pf

            p_loc, p_full = alloc_inter(0)

            # ---- constants
            w1_sb = cpool.tile([P, 4, 512], f16)
            nc.sync.dma_start(w1_sb[:], w1_d[:])
            w2_sb = cpool.tile([P, 2, 512], f16)
            nc.sync.dma_start(w2_sb[:], w2_d[:])
            w3_sb = cpool.tile([P, 2, 512], f16)
            nc.sync.dma_start(w3_sb[:], w3_d[:])
            wl_sb = cpool.tile([P, 6, NCLS], f16)
            nc.sync.dma_start(wl_sb[:], wl_d[:])
            b_sb = [None]
            for name, t in (("b2", b2_d), ("b3", b3_d)):
                bt = cpool.tile([P, HID], f16, name=name + "sb")
                nc.sync.dma_start(bt[:], t[:])
                b_sb.append(bt)
            bl_sb = cpool.tile([P, NCLS], f16)
            nc.sync.dma_start(bl_sb[:], bl_d[:])

            meta_sb = cpool.tile([P, TPC, 2 * K], f16)
            nc.sync.dma_start(
                meta_sb[:],
                meta_dw_d[:].rearrange("t p c -> p t c"),
            )
            gidx_sb = cpool.tile([P, NGRP, GI_COLS], i16)
            nc.sync.dma_start(
                gidx_sb[:],
                gidx_d[:].rearrange("g p c -> p g c"),
            )

            iota_i = cpool.tile([P, P], i32)
            nc.gpsimd.iota(iota_i[:], pattern=[[1, P]], base=0, channel_multiplier=0)
            iota_f = cpool.tile([P, P], f16)
            nc.vector.tensor_copy(iota_f[:], iota_i[:])
            ident = cpool.tile([P, P], f16)
            make_identity(nc, ident[:])

            x1t_sb = cpool.tile([P, TPC, 2, P], f16)
            # scratch: layer-1 r (as [P, t, 256]) then layer-2 x^T blocks
            scr = cpool.tile([P, TPC, 2, P], f16)
            out_sb = cpool.tile([P, TPC, NCLS], f32)

            def scr_r(t):
                return scr[:, t].rearrange("p h q -> p (h q)")

            def wtop(layer):
                return (w2_sb if layer == 1 else w3_sb)

            # ---- phase A: p (and r for layer 1) per 4-tile group
            def grp_rows(buf, g):
                return buf[g * G * P:(g + 1) * G * P, :].rearrange(
                    "(t p) c -> p t c", p=P
                )

            def phase_a(layer):
                for g in range(NGRP):
                    if layer == 0:
                        lxs = []
                        for k in range(4):
                            lx = lxpool.tile([P, G * P], f16, name="lx", tag="lx")
                            nc.sync.dma_start(
                                lx[:],
                                xTc[k * P:(k + 1) * P, g * G * P:(g + 1) * G * P],
                            )
                            lxs.append(lx)
                    pst = stpool.tile([P, G, HID], f16, name="pst", tag="pst")
                    for gt in range(G):
                        t = g * G + gt
                        if layer == 0:
                            ps = psa.tile([P, 512], f32, name="psA", tag="psA")
                            for k in range(4):
                                nc.tensor.matmul(
                                    out=ps[:], lhsT=lxs[k][:, gt * P:(gt + 1) * P],
                                    rhs=w1_sb[:, k, :],
                                    start=(k == 0), stop=(k == 3),
                                )
                            nc.scalar.activation(
                                scr_r(t), ps[:, :HID],
                                mybir.ActivationFunctionType.Copy,
                            )
                            nc.vector.tensor_copy(pst[:, gt, :], ps[:, HID:])
                        else:
                            xsrc = x1t_sb if layer == 1 else scr
                            ps = psa.tile([P, HID], f32, name="psA2", tag="psA")
                            for k in range(2):
                                nc.tensor.matmul(
                                    out=ps[:], lhsT=xsrc[:, t, k, :],
                                    rhs=wtop(layer)[:, k, HID:],
                                    start=(k == 0), stop=(k == 1),
                                )
                            nc.vector.tensor_copy(pst[:, gt, :], ps[:])
                    nc.sync.dma_start(grp_rows(p_loc[layer], g), pst[:])

            # ---- phase B: aggregate + relu (+ classifier on last layer)
            def phase_b(layer, sub="full"):
                last = layer == 2
                qrr = [0]
                for g in range(NGRP):
                    msgs = []
                    col = 0
                    for w in range(NW):
                        nch = G * Kw[w]
                        m = msgpool.tile(
                            [P, nch, HID], f16, name=f"m{w}", tag=f"m{w}"
                        )
                        # split into <=1024-idx calls (64-desc packets/engine),
                        # round-robined over the 4 SWDGE queues
                        for c0 in range(0, nch, 8):
                            c1 = min(c0 + 8, nch)
                            nidx = (c1 - c0) * P
                            nc.gpsimd.dma_gather(
                                out_ap=m[:, c0:c1, :],
                                in_ap=p_full[layer][WBASE[w]:WBASE[w] + WCAP, :],
                                idxs_ap=gidx_sb[
                                    :, g, col + c0 * 8:col + c1 * 8
                                ],
                                num_idxs=nidx,
                                num_idxs_reg=nidx,
                                elem_size=HID,
                                single_packet=True,
                                queue_num=qrr[0] % 4,
                            )
                            qrr[0] += 1
                        msgs.append(m)
                        col += nch * 8
                    if last:
                        shst = stpool.tile([P, G, NCLS], f32, name="shst", tag="shst")
                        smst = stpool.tile([P, G], f32, name="smst", tag="smst")
                    for gt in range(G):
                        t = g * G + gt
                        if sub == "gather":
                            nc.vector.tensor_copy(
                                out_sb[:, t, :], msgs[0][:, gt * Kw[0], :NCLS]
                            )
                            continue
                        s_all = sbpool.tile([P, K, P], f16, name="sall", tag="sall")
                        nc.vector.tensor_tensor(
                            out=s_all[:],
                            in0=meta_sb[:, t, :K].unsqueeze(2).broadcast_to([P, K, P]),
                            in1=iota_f[:].unsqueeze(1).broadcast_to([P, K, P]),
                            op=mybir.AluOpType.is_equal,
                        )
                        nc.vector.tensor_tensor(
                            out=s_all[:],
                            in0=s_all[:],
                            in1=meta_sb[:, t, K:2 * K].unsqueeze(2).broadcast_to(
                                [P, K, P]
                            ),
                            op=mybir.AluOpType.mult,
                        )
                        agg = psagg.tile([P, HID], f32, name="agg", tag="agg")
                        nmm = 1 + (0 if layer == 0 else 2) + K
                        c = 0
                        nc.tensor.matmul(
                            out=agg[:], lhsT=ident[:],
                            rhs=(scr_r(t) if layer == 0 else b_sb[layer][:]),
                            start=True, stop=(nmm == 1),
                        )
                        c += 1
                        if layer > 0:
                            xsrc = x1t_sb if layer == 1 else scr
                            for k in range(2):
                                nc.tensor.matmul(
                                    out=agg[:], lhsT=xsrc[:, t, k, :],
                                    rhs=wtop(layer)[:, k, :HID],
                                    start=False, stop=False,
                                )
                                c += 1
                        for w in range(NW):
                            for j in range(Kw[w]):
                                nc.tensor.matmul(
                                    out=agg[:],
                                    lhsT=s_all[:, offw[w] + j, :],
                                    rhs=msgs[w][:, gt * Kw[w] + j, :],
                                    start=False,
                                    stop=(c == nmm - 1),
                                )
                                c += 1
                        xn = stpool.tile([P, HID], f16, name="xn", tag="xn")
                        nc.scalar.activation(
                            xn[:], agg[:], mybir.ActivationFunctionType.Relu
                        )
                        x3t = []
                        for h in range(2):
                            tp = pstr.tile([P, P], f16, name="tp", tag="tp")
                            nc.tensor.transpose(
                                out=tp[:], in_=xn[:, h * P:(h + 1) * P],
                                identity=ident[:],
                            )
                            if layer == 0:
                                nc.scalar.activation(
                                    x1t_sb[:, t, h, :], tp[:],
                                    mybir.ActivationFunctionType.Copy,
                                )
                            elif layer == 1:
                                nc.scalar.activation(
                                    scr[:, t, h, :], tp[:],
                                    mybir.ActivationFunctionType.Copy,
                                )
                            else:
                                xt = stpool.tile([P, P], f16, name="x3t", tag="x3t")
                                nc.vector.tensor_copy(xt[:], tp[:])
                                x3t.append(xt)
                        if last:
                            # classifier: 6 k-chunks of 128 + folded bias
                            cls = pscls.tile([P, NCLS], f32, name="cls", tag="cls")
                            chunks = [
                                x1t_sb[:, t, 0, :], x1t_sb[:, t, 1, :],
                                scr[:, t, 0, :], scr[:, t, 1, :],
                                x3t[0][:], x3t[1][:],
                            ]
                            nc.tensor.matmul(
                                out=cls[:], lhsT=ident[:], rhs=bl_sb[:],
                                start=True, stop=False,
                            )
                            for kk in range(6):
                                nc.tensor.matmul(
                                    out=cls[:], lhsT=chunks[kk], rhs=wl_sb[:, kk, :],
                                    start=False, stop=(kk == 5),
                                )
                            mx = stpool.tile([P, 1], f32, name="mx", tag="mx")
                            nc.vector.tensor_reduce(
                                out=mx[:], in_=cls[:], axis=mybir.AxisListType.X,
                                op=mybir.AluOpType.max,
                            )
                            nc.vector.tensor_scalar(
                                out=shst[:, gt, :], in0=cls[:], scalar1=mx[:, :1],
                                scalar2=None, op0=mybir.AluOpType.subtract,
                            )
                    if last and sub == "full":
                        ex = stpool.tile([P, G, NCLS], f32, name="ex", tag="ex")
                        nc.scalar.activation(
                            ex[:], shst[:], mybir.ActivationFunctionType.Exp
                        )
                        nc.vector.tensor_reduce(
                            out=smst[:], in_=ex[:], axis=mybir.AxisListType.X,
                            op=mybir.AluOpType.add,
                        )
                        lsm = stpool.tile([P, G], f32, name="lsm", tag="lsm")
                        nc.scalar.activation(
                            lsm[:], smst[:], mybir.ActivationFunctionType.Ln
                        )
                        for gt in range(G):
                            t = g * G + gt
                            nc.vector.tensor_scalar(
                                out=out_sb[:, t, :], in0=shst[:, gt, :],
                                scalar1=lsm[:, gt:gt + 1],
                                scalar2=None, op0=mybir.AluOpType.subtract,
                            )

            if stage != "full":
                nc.gpsimd.memset(out_sb[:], 0.0)
            nlayers = {"a1": 1, "ag1": 1, "b1": 1, "b1g": 3}.get(stage, 3)
            bsub = {"b1g": "gather"}.get(stage, "full")
            do_ag = stage not in ("a1",)
            do_b = stage not in ("a1", "ag1")
            if not do_b or bsub != "full":
                # timing-only variants: initialize tensors phase B would write
                nc.gpsimd.memset(x1t_sb[:], 0.0)
                nc.gpsimd.memset(scr[:], 0.0)
            for _rep in range(repeat):
                if _rep > 0:
                    p_loc, p_full = alloc_inter(_rep)
                for layer in range(nlayers):
                    phase_a(layer)
                    if do_ag:
                        nc.gpsimd.collective_compute(
                            "AllGather",
                            mybir.AluOpType.bypass,
                            replica_groups=[list(range(NC))],
                            ins=[p_loc[layer].opt()],
                            outs=[p_full[layer].opt()],
                        )
                    if do_b:
                        phase_b(layer, sub=bsub)

            nc.sync.dma_start(
                out_d[:].rearrange("(t p) j -> p t j", p=P), out_sb[:]
            )

    nc.compile()
    return nc


# --------------------------------------------------------------------------
# entry point
# --------------------------------------------------------------------------

def make_in_maps(pp, wts):
    in_maps = []
    for c in range(NC):
        in_maps.append({
            "xTc": np.ascontiguousarray(pp["xT"][:, c * NPC:(c + 1) * NPC]),
            "meta_dw": np.ascontiguousarray(pp["meta_dw"][c * TPC:(c + 1) * TPC]),
            "gidx": np.ascontiguousarray(pp["gidx"][c * NGRP:(c + 1) * NGRP]),
            **wts,
        })
    return in_maps


def kernel(x, edge_index, edge_weight, W1, b1, W2, b2, W3, b3, Wl, bl):
    x = np.asarray(x, dtype=np.float32)
    edge_index = np.asarray(edge_index)
    edge_weight = np.asarray(edge_weight, dtype=np.float32)

    pp = prep(x, edge_index, edge_weight)
    K, Kw = pp["K"], pp["Kw"]
    wts = pack_weights(
        np.asarray(W1, np.float32), np.asarray(b1, np.float32),
        np.asarray(W2, np.float32), np.asarray(b2, np.float32),
        np.asarray(W3, np.float32), np.asarray(b3, np.float32),
        np.asarray(Wl, np.float32), np.asarray(bl, np.float32),
    )

    key = (K, Kw)
    if key not in _compile_cache:
        _compile_cache[key] = build(K, Kw)
    nc = _compile_cache[key]

    in_maps = make_in_maps(pp, wts)

    res = run_bass_kernel_spmd(nc, in_maps, list(range(NC)))
    out_full = np.concatenate([res.results[c]["out"] for c in range(NC)], axis=0)
    return out_full[pp["newpos"][:N]].astype(np.float32)


if __name__ == "__main__":
    import time

    rng = np.random.default_rng(0)
    E = 899756
    ei = rng.integers(0, N, (2, E)).astype(np.int32)
    ew = rng.random(E, dtype=np.float32)
    x = rng.standard_normal((N, IN_F), dtype=np.float32)
    t0 = time.time()
    pp = prep(x, ei, ew)
    print("prep", time.time() - t0, "K =", pp["K"], "Kw =", pp["Kw"])
